# revision 16
# baseline (speedup 1.0000x reference)
"""ConvAttention Trainium2 kernel (8-core data-parallel over batch).

Reference computation (per batch b):
  k  = relu(conv1d(keys, kw1, kb1, pad=1)); k = conv1d(k, kw2, kb2)   # (80, 200)
  q  = relu(conv1d(queries, qw1, qb1, pad=1)); q = relu(conv1d(q, qw2, qb2))
  q  = conv1d(q, qw3, qb3)                                            # (80, 800)
  s  = -0.0005*(q2 + k2 - 2 qk);  out = log_softmax(s, T2) + log(prior + 1e-8)

Kernel algebra:
  - q2 (constant along the T2 softmax axis) cancels in log_softmax: never computed.
  - 0.001 folded into qw3/qb3 on host; the attention matmul uses an augmented
    contraction row (row 96 of lhsT = 1, row 96 of rhs = -0.0005*k2, rows
    80..95 zero) so a single matmul yields s directly.
  - softmax without max subtraction (|s| <~ 0.2, exp is safe in f32).

Engine split: PE matmuls; ACT only Exp/Ln (one activation-table set);
DVE for PSUM epilogues / copies / reductions; GpSimd for SBUF-only ops.
Inputs arrive in two packed bf16 blobs (weights + activations + bitcast f32
biases) to minimize per-DMA fixed cost.

Sharding: batch 16 -> 2 per core x 8 cores. No collectives.
"""

import os
import sys

for _p in ("/opt/trn_rl_repo",):
    if _p not in sys.path:
        sys.path.append(_p)

import numpy as np
import ml_dtypes

import concourse.bass as bass
import concourse.tile as tile
from concourse import mybir
import bass_rust
from concourse.bass_utils import run_bass_kernel_spmd

BF16 = ml_dtypes.bfloat16
F32 = mybir.dt.float32
BF = mybir.dt.bfloat16

N_CORES = 8
B, CMEL, CTXT, CATT, T1, T2 = 16, 80, 256, 80, 800, 200
BL = B // N_CORES          # 2 batches per core
P1 = 100                   # t1-tile partition rows
NT1 = T1 // P1             # 8 t1 tiles per batch
AF = mybir.ActivationFunctionType
ALU = mybir.AluOpType
AX = mybir.AxisListType

# kblob bf16 column offsets
K_W1, K_W2, K_KX, K_BIAS = 0, 3072, 3392, 4200
K_C = K_BIAS + 22
# f32 columns inside the bitcast bias region
BC_B1, BC_B1Q, BC_B2, BC_B2Q, BC_B3Q, BC_ONES, BC_EPS = 0, 4, 6, 7, 8, 9, 10
# qblob bf16 column offsets
Q_W1, Q_W2, Q_W3, Q_QX = 0, 480, 640, 720
Q_C = Q_QX + BL * (T1 + 2)


def _split_multi_waits(nc):
    """This walrus build accepts at most one semaphore wait per instruction.
    Hoist extra waits onto standalone EventSemaphore instructions placed
    immediately before the owner (same engine, program order preserved)."""
    for f in nc.m.functions:
        for bb in f.blocks:
            out, changed = [], False
            for inst in list(bb.instructions):
                si = inst.sync_info
                if si is not None and si.on_wait is not None and len(si.on_wait) > 1:
                    waits = list(si.on_wait)
                    for j, w in enumerate(waits[:-1]):
                        out.append(mybir.InstEventSemaphore(
                            name=f"{inst.name}-hw{j}", engine=inst.engine,
                            sync_info=bass_rust.SyncInfo(on_wait=[w], on_update=[])))
                    si.on_wait = [waits[-1]]
                    changed = True
                out.append(inst)
            if changed:
                bb.instructions = out


def _build(fixup=True, loop_k=0):
    import contextlib
    nc = bass.Bass()

    kblob = nc.dram_tensor("kblob", (128, K_C), BF, kind="ExternalInput")
    qblob = nc.dram_tensor("qblob", (128, Q_C), BF, kind="ExternalInput")
    p_x = nc.dram_tensor("p_x", (P1, BL, NT1, T2), BF, kind="ExternalInput")
    out_l = nc.dram_tensor("out_l", (BL, P1, NT1, T2), F32, kind="ExternalOutput")

    with tile.TileContext(nc) as tc:
        with (
            tc.tile_pool(name="wts", bufs=1) as wts,
            tc.tile_pool(name="enc", bufs=1) as enc,
            tc.tile_pool(name="att", bufs=2) as att,
            tc.tile_pool(name="pc", bufs=2, space="PSUM") as pc,
            tc.tile_pool(name="pa", bufs=3, space="PSUM") as pa,
            contextlib.ExitStack() as _loop_ctx,
        ):
            if loop_k:
                _loop_ctx.enter_context(tc.For_i(0, loop_k, 1))
            kb = wts.tile([128, K_C], BF)
            qb = wts.tile([128, Q_C], BF)
            nc.sync.dma_start(kb[:], kblob[:])
            nc.sync.dma_start(qb[:], qblob[:])
            bia = kb[:, K_BIAS:K_C].bitcast(F32)            # (128, 11) f32
            kx = kb[:, K_KX:K_BIAS].rearrange(
                "p (c b t) -> p c b t", c=2, b=BL)          # (128, 2, BL, 202)
            qx = qb[:, Q_QX:Q_C].rearrange(
                "p (b t) -> p b t", b=BL)                   # (128, BL, 802)

            def w1_at(q, m):
                return kb[:, K_W1 + q * 512 + m * 128: K_W1 + q * 512 + (m + 1) * 128]

            # ---- key encoder (both batches per matmul via 3D rhs) ----
            k1 = enc.tile([128, 4, BL, T2], BF)
            for m in range(4):
                psk = pc.tile([128, BL, T2], F32, tag="pc")
                for j in range(6):
                    dk, cik = j // 2, j % 2
                    nc.tensor.matmul(
                        psk[:], w1_at(dk * 2 + cik, m),
                        kx[:, cik, :, dk:dk + T2],
                        start=(j == 0), stop=(j == 5))
                nc.scalar.activation(k1[:, m, :, :], psk[:], AF.Relu,
                                     bias=bia[:, BC_B1 + m:BC_B1 + m + 1])

            psk2 = pc.tile([80, BL, T2], F32, tag="pc")
            for m in range(4):
                nc.tensor.matmul(psk2[:], kb[:, K_W2 + m * 80:K_W2 + (m + 1) * 80],
                                 k1[:, m, :, :], start=(m == 0), stop=(m == 3))
            k_f = enc.tile([80, BL, T2], F32)
            ksq = enc.tile([80, BL, T2], F32)
            # rows 0..79 = k, rows 80..95 = 0, row 96 = -5e-4*k2
            # (single-row writes must start at a 32-aligned partition)
            k_sb = enc.tile([97, BL, T2], BF)
            nc.gpsimd.memset(k_sb[:], 0.0)
            nc.vector.tensor_scalar_add(k_f[:], psk2[:], bia[0:80, BC_B2:BC_B2 + 1])
            nc.gpsimd.tensor_copy(k_sb[0:80, :, :], k_f[:])
            nc.vector.tensor_mul(ksq[:], k_f[:], k_f[:])
            psk3 = pc.tile([1, BL, T2], F32, tag="pc")
            nc.tensor.matmul(psk3[:], bia[0:80, BC_ONES:BC_ONES + 1], ksq[:],
                             start=True, stop=True)
            nc.vector.tensor_scalar_mul(k_sb[96:97, :, :], psk3[:], -0.0005)

            # ---- query encoder ----
            q1 = enc.tile([80, 2, BL, T1], BF)          # (h-chunk, b, T1)
            q2t = enc.tile([80, BL, T1], BF)
            q_aug = enc.tile([97, BL, T1], BF)          # rows 80..95 = 0, row 96 = 1
            nc.gpsimd.memset(q_aug[:], 0.0)
            nc.gpsimd.memset(q_aug[96:97, :, :], 1.0)
            NQ = 400
            for b in range(BL):
                for n in range(2):
                    sl = slice(n * NQ, (n + 1) * NQ)
                    for h in range(2):
                        psq = pc.tile([80, NQ], F32, tag="pc")
                        for dk in range(3):
                            nc.tensor.matmul(
                                psq[:],
                                qb[0:80, Q_W1 + dk * 160 + h * 80:
                                   Q_W1 + dk * 160 + (h + 1) * 80],
                                qx[0:80, b, dk + n * NQ: dk + n * NQ + NQ],
                                start=(dk == 0), stop=(dk == 2))
                        nc.scalar.activation(
                            q1[:, h, b, sl], psq[:], AF.Relu,
                            bias=bia[0:80, BC_B1Q + h:BC_B1Q + h + 1])
                    psq2 = pc.tile([80, NQ], F32, tag="pc")
                    for h in range(2):
                        nc.tensor.matmul(
                            psq2[:], qb[0:80, Q_W2 + h * 80:Q_W2 + (h + 1) * 80],
                            q1[:, h, b, sl], start=(h == 0), stop=(h == 1))
                    nc.vector.tensor_scalar(
                        q2t[:, b, sl], psq2[:],
                        scalar1=bia[0:80, BC_B2Q:BC_B2Q + 1], scalar2=0.0,
                        op0=ALU.add, op1=ALU.max)
                    psq3 = pc.tile([80, NQ], F32, tag="pc")
                    nc.tensor.matmul(psq3[:], qb[0:80, Q_W3:Q_W3 + 80],
                                     q2t[:, b, sl], start=True, stop=True)
                    nc.vector.tensor_scalar_add(q_aug[0:80, b, sl], psq3[:],
                                                bia[0:80, BC_B3Q:BC_B3Q + 1])

            # ---- attention + log_softmax + log prior ----
            p_t = enc.tile([P1, BL, NT1, T2], BF)
            lp = enc.tile([P1, BL, NT1, T2], F32)
            nc.sync.dma_start(p_t[:], p_x[:])
            nc.scalar.activation(lp[:], p_t[:], AF.Ln, bias=bia[0:P1, BC_EPS:BC_EPS + 1])
            for b in range(BL):
                s_sb = att.tile([P1, NT1, T2], BF, tag="s_sb")
                e_sb = att.tile([P1, NT1, T2], BF, tag="e_sb")
                sume = att.tile([P1, NT1], F32, tag="sume")
                lse = att.tile([P1, NT1], F32, tag="lse")
                obig = att.tile([P1, NT1, T2], F32, tag="obig")
                for g in range(4):
                    pst = pa.tile([P1, 2, 512], F32, tag="pa")
                    for j in range(2):
                        i = g * 2 + j
                        nc.tensor.matmul(pst[:, j, 0:T2],
                                         q_aug[:, b, i * P1:(i + 1) * P1],
                                         k_sb[:, b, :], start=True, stop=True)
                    nc.vector.tensor_copy(s_sb[:, 2 * g:2 * g + 2, :],
                                          pst[:, :, 0:T2])
                nc.scalar.activation(e_sb[:], s_sb[:], AF.Exp)
                nc.vector.reduce_sum(sume[:], e_sb[:], axis=AX.X)
                nc.scalar.activation(lse[:], sume[:], AF.Ln)
                for i in range(NT1):
                    nc.vector.scalar_tensor_tensor(
                        obig[:, i, :], in0=s_sb[:, i, :], scalar=lse[:, i:i + 1],
                        in1=lp[:, b, i, :], op0=ALU.subtract, op1=ALU.add)
                nc.sync.dma_start(out_l[b], obig[:])

    if fixup:
        _split_multi_waits(nc)
    return nc


_NC = None
_last_res = None


def _get_nc():
    global _NC
    if _NC is None:
        _NC = _build()
    return _NC


def _pack_weights(kw1, kb1, kw2, kb2, qw1, qb1, qw2, qb2, qw3, qb3):
    """Shared (per-core-independent) blob regions."""
    kw = np.zeros((128, K_C), BF16)
    kw[:, K_W1:K_W2] = (
        kw1.transpose(1, 2, 0).reshape(2, 128, 3, 512)
        .transpose(1, 2, 0, 3).reshape(128, 3072).astype(BF16))
    kw[:, K_W2:K_KX] = (
        kw2[:, :, 0].T.reshape(4, 128, 80).transpose(1, 0, 2)
        .reshape(128, 320).astype(BF16))
    bias = np.zeros((128, 11), np.float32)
    bias[:, BC_B1:BC_B1 + 4] = kb1.reshape(4, 128).T
    bias[0:80, BC_B1Q:BC_B1Q + 2] = qb1.reshape(2, 80).T
    bias[0:80, BC_B2] = kb2
    bias[0:80, BC_B2Q] = qb2
    bias[0:80, BC_B3Q] = 1e-3 * qb3
    bias[:, BC_ONES] = 1.0
    bias[:, BC_EPS] = 1e-8
    kw[:, K_BIAS:K_C] = bias.view(BF16)

    qw = np.zeros((128, Q_C), BF16)
    qw[0:80, Q_W1:Q_W2] = (
        qw1.transpose(1, 2, 0).reshape(80, 480).astype(BF16))
    qw[0:80, Q_W2:Q_W3] = (
        qw2[:, :, 0].T.reshape(2, 80, 80).transpose(1, 0, 2)
        .reshape(80, 160).astype(BF16))
    qw[0:80, Q_W3:Q_W3 + 80] = (1e-3 * qw3[:, :, 0]).T.astype(BF16)
    return kw, qw


def _prep_core(queries, keys, prior, kw, qw):
    kblob = kw.copy()
    kxr = np.zeros((2, 128, BL, T2 + 2), BF16)
    kxr[:, :, :, 1:T2 + 1] = (
        keys.reshape(BL, 2, 128, T2).transpose(1, 2, 0, 3).astype(BF16))
    kblob[:, K_KX:K_BIAS] = kxr.transpose(1, 0, 2, 3).reshape(128, 808)

    qblob = qw.copy()
    qxr = np.zeros((80, BL, T1 + 2), BF16)
    qxr[:, :, 1:T1 + 1] = queries.transpose(1, 0, 2).astype(BF16)
    qblob[0:80, Q_QX:Q_C] = qxr.reshape(80, BL * (T1 + 2))

    p_x = np.ascontiguousarray(
        prior.reshape(BL, NT1, P1, T2).transpose(2, 0, 1, 3).astype(BF16))
    return {"kblob": kblob, "qblob": qblob, "p_x": p_x}


def kernel(queries, keys, attn_prior, kw1, kb1, kw2, kb2,
           qw1, qb1, qw2, qb2, qw3, qb3):
    nc = _get_nc()
    kw, qw = _pack_weights(
        np.asarray(kw1, np.float32), np.asarray(kb1, np.float32),
        np.asarray(kw2, np.float32), np.asarray(kb2, np.float32),
        np.asarray(qw1, np.float32), np.asarray(qb1, np.float32),
        np.asarray(qw2, np.float32), np.asarray(qb2, np.float32),
        np.asarray(qw3, np.float32), np.asarray(qb3, np.float32))
    queries = np.asarray(queries, np.float32)
    keys = np.asarray(keys, np.float32)
    attn_prior = np.asarray(attn_prior, np.float32)

    in_maps = [
        _prep_core(queries[c * BL:(c + 1) * BL], keys[c * BL:(c + 1) * BL],
                   attn_prior[c * BL:(c + 1) * BL], kw, qw)
        for c in range(N_CORES)
    ]
    trace = bool(os.environ.get("CONVATTN_TRACE"))
    res = run_bass_kernel_spmd(nc, in_maps, core_ids=list(range(N_CORES)),
                               trace=trace)
    global _last_res
    _last_res = res

    full = np.empty((B, T1, T2), np.float32)
    for c in range(N_CORES):
        o = res.results[c]["out_l"]          # (BL, P1, NT1, T2)
        full[c * BL:(c + 1) * BL] = (
            o.transpose(0, 2, 1, 3).reshape(BL, T1, T2))
    return full[:, None]


# revision 18
# speedup vs baseline: 2.1885x; 2.1885x over previous
"""ConvAttention Trainium2 kernel (8-core data-parallel over batch).

Reference computation (per batch b):
  k  = relu(conv1d(keys, kw1, kb1, pad=1)); k = conv1d(k, kw2, kb2)   # (80, 200)
  q  = relu(conv1d(queries, qw1, qb1, pad=1)); q = relu(conv1d(q, qw2, qb2))
  q  = conv1d(q, qw3, qb3)                                            # (80, 800)
  s  = -0.0005*(q2 + k2 - 2 qk);  out = log_softmax(s, T2) + log(prior + 1e-8)

Kernel algebra:
  - q2 (constant along the T2 softmax axis) cancels in log_softmax: never computed.
  - 0.001 folded into qw3/qb3 on host; the attention matmul uses an augmented
    contraction row (row 96 of lhsT = 1, row 96 of rhs = -0.0005*k2, rows
    80..95 zero) so a single matmul yields s directly.
  - softmax without max subtraction (|s| <~ 0.2, exp is safe in f32).

Engine split: PE matmuls; ACT relu epilogues + Exp/Ln; DVE PSUM epilogues /
copies / reductions; GpSimd memsets + SBUF-only copies.
Inputs arrive in two packed bf16 blobs (weights + activations + bitcast f32
biases) plus the prior, to minimize per-DMA fixed cost. Output is bf16,
upcast to f32 on the host.

Sharding: batch 16 -> 2 per core x 8 cores. No collectives.
"""

import contextlib
import os
import sys

for _p in ("/opt/trn_rl_repo",):
    if _p not in sys.path:
        sys.path.append(_p)

import numpy as np
import ml_dtypes

import concourse.bass as bass
import concourse.tile as tile
from concourse import mybir
import bass_rust
from concourse.bass_utils import run_bass_kernel_spmd

BF16 = ml_dtypes.bfloat16
F32 = mybir.dt.float32
BF = mybir.dt.bfloat16

N_CORES = 8
B, CMEL, CTXT, CATT, T1, T2 = 16, 80, 256, 80, 800, 200
BL = B // N_CORES          # 2 batches per core
P1 = 100                   # t1-tile partition rows
NT1 = T1 // P1             # 8 t1 tiles per batch
AF = mybir.ActivationFunctionType
ALU = mybir.AluOpType
AX = mybir.AxisListType

# kblob bf16 column offsets: key-conv weights + padded keys
K_W1, K_W2, K_KX = 0, 3072, 3392
K_C = K_KX + 2 * BL * (T2 + 2)
# qblob bf16 column offsets: query weights + f32 bias region + padded queries
Q_W1, Q_W2, Q_W3, Q_BIAS = 0, 480, 640, 720
Q_QX = Q_BIAS + 22
Q_C = Q_QX + BL * (T1 + 2)
# f32 columns inside the bitcast bias region
BC_B1, BC_B1Q, BC_B2, BC_B2Q, BC_B3Q, BC_ONES, BC_EPS = 0, 4, 6, 7, 8, 9, 10


def _split_multi_waits(nc):
    """This walrus build accepts at most one semaphore wait per instruction.
    Hoist extra waits onto standalone EventSemaphore instructions placed
    immediately before the owner (same engine, program order preserved)."""
    for f in nc.m.functions:
        for bb in f.blocks:
            out, changed = [], False
            for inst in list(bb.instructions):
                si = inst.sync_info
                if si is not None and si.on_wait is not None and len(si.on_wait) > 1:
                    waits = list(si.on_wait)
                    for j, w in enumerate(waits[:-1]):
                        out.append(mybir.InstEventSemaphore(
                            name=f"{inst.name}-hw{j}", engine=inst.engine,
                            sync_info=bass_rust.SyncInfo(on_wait=[w], on_update=[])))
                    si.on_wait = [waits[-1]]
                    changed = True
                out.append(inst)
            if changed:
                bb.instructions = out


def _build(fixup=True, loop_k=0):
    nc = bass.Bass()

    qblob = nc.dram_tensor("qblob", (128, Q_C), BF, kind="ExternalInput")
    kblob = nc.dram_tensor("kblob", (128, K_C), BF, kind="ExternalInput")
    p_x = nc.dram_tensor("p_x", (P1, BL, NT1, T2), BF, kind="ExternalInput")
    out_l = nc.dram_tensor("out_l", (BL, P1, NT1, T2), BF, kind="ExternalOutput")

    with tile.TileContext(nc) as tc:
        with (
            tc.tile_pool(name="wts", bufs=1) as wts,
            tc.tile_pool(name="enc", bufs=1) as enc,
            tc.tile_pool(name="att", bufs=2) as att,
            tc.tile_pool(name="pc", bufs=4, space="PSUM") as pc,
            tc.tile_pool(name="pa", bufs=2, space="PSUM") as pa,
            contextlib.ExitStack() as _loop_ctx,
        ):
            if loop_k:
                _loop_ctx.enter_context(tc.For_i(0, loop_k, 1))
            qb = wts.tile([128, Q_C], BF)
            kb = wts.tile([128, K_C], BF)
            p_t = enc.tile([P1, BL, NT1, T2], BF)
            nc.sync.dma_start(qb[:], qblob[:])
            nc.sync.dma_start(kb[:], kblob[:])
            nc.sync.dma_start(p_t[:], p_x[:])
            bia = qb[:, Q_BIAS:Q_QX].bitcast(F32)           # (128, 11) f32
            kx = kb[:, K_KX:K_C].rearrange(
                "p (c b t) -> p c b t", c=2, b=BL)          # (128, 2, BL, 202)
            qx = qb[:, Q_QX:Q_C].rearrange(
                "p (b t) -> p b t", b=BL)                   # (128, BL, 802)

            # ---- query encoder (per batch; b=0 first so attention starts early)
            q1 = enc.tile([80, 2, BL, T1], BF)          # (h-chunk, b, T1)
            q2t = enc.tile([80, BL, T1], BF)
            q_aug = enc.tile([97, BL, T1], BF)          # rows 80..95 = 0, row 96 = 1
            nc.gpsimd.memset(q_aug[:], 0.0)
            nc.gpsimd.memset(q_aug[96:97, :, :], 1.0)
            NQ = 400

            def query_encoder(b):
                for n in range(2):
                    sl = slice(n * NQ, (n + 1) * NQ)
                    for h in range(2):
                        psq = pc.tile([80, NQ], F32, tag="pc")
                        for dk in range(3):
                            nc.tensor.matmul(
                                psq[:],
                                qb[0:80, Q_W1 + dk * 160 + h * 80:
                                   Q_W1 + dk * 160 + (h + 1) * 80],
                                qx[0:80, b, dk + n * NQ: dk + n * NQ + NQ],
                                start=(dk == 0), stop=(dk == 2))
                        nc.scalar.activation(
                            q1[:, h, b, sl], psq[:], AF.Relu,
                            bias=bia[0:80, BC_B1Q + h:BC_B1Q + h + 1])
                    psq2 = pc.tile([80, NQ], F32, tag="pc")
                    for h in range(2):
                        nc.tensor.matmul(
                            psq2[:], qb[0:80, Q_W2 + h * 80:Q_W2 + (h + 1) * 80],
                            q1[:, h, b, sl], start=(h == 0), stop=(h == 1))
                    nc.vector.tensor_scalar(
                        q2t[:, b, sl], psq2[:],
                        scalar1=bia[0:80, BC_B2Q:BC_B2Q + 1], scalar2=0.0,
                        op0=ALU.add, op1=ALU.max)
                    psq3 = pc.tile([80, NQ], F32, tag="pc")
                    nc.tensor.matmul(psq3[:], qb[0:80, Q_W3:Q_W3 + 80],
                                     q2t[:, b, sl], start=True, stop=True)
                    nc.vector.tensor_scalar_add(q_aug[0:80, b, sl], psq3[:],
                                                bia[0:80, BC_B3Q:BC_B3Q + 1])

            query_encoder(0)

            # ---- key encoder (both batches per matmul via 3D rhs) ----
            k1 = enc.tile([128, 4, BL, T2], BF)
            for m in range(4):
                psk = pc.tile([128, BL, T2], F32, tag="pc")
                for j in range(6):
                    dk, cik = j // 2, j % 2
                    nc.tensor.matmul(
                        psk[:],
                        kb[:, K_W1 + (dk * 2 + cik) * 512 + m * 128:
                           K_W1 + (dk * 2 + cik) * 512 + (m + 1) * 128],
                        kx[:, cik, :, dk:dk + T2],
                        start=(j == 0), stop=(j == 5))
                nc.scalar.activation(k1[:, m, :, :], psk[:], AF.Relu,
                                     bias=bia[:, BC_B1 + m:BC_B1 + m + 1])

            psk2 = pc.tile([80, BL, T2], F32, tag="pc")
            for m in range(4):
                nc.tensor.matmul(psk2[:], kb[:, K_W2 + m * 80:K_W2 + (m + 1) * 80],
                                 k1[:, m, :, :], start=(m == 0), stop=(m == 3))
            k_f = enc.tile([80, BL, T2], F32)
            ksq = enc.tile([80, BL, T2], F32)
            # rows 0..79 = k, rows 80..95 = 0, row 96 = -5e-4*k2
            # (single-row writes must start at a 32-aligned partition)
            k_sb = enc.tile([97, BL, T2], BF)
            nc.gpsimd.memset(k_sb[:], 0.0)
            nc.vector.tensor_scalar_add(k_f[:], psk2[:], bia[0:80, BC_B2:BC_B2 + 1])
            nc.gpsimd.tensor_copy(k_sb[0:80, :, :], k_f[:])
            nc.vector.tensor_mul(ksq[:], k_f[:], k_f[:])
            psk3 = pc.tile([1, BL, T2], F32, tag="pc")
            nc.tensor.matmul(psk3[:], bia[0:80, BC_ONES:BC_ONES + 1], ksq[:],
                             start=True, stop=True)
            nc.vector.tensor_scalar_mul(k_sb[96:97, :, :], psk3[:], -0.0005)

            # ---- attention + log_softmax + log prior ----
            lp = enc.tile([P1, BL, NT1, T2], F32)
            nc.scalar.activation(lp[:], p_t[:], AF.Ln,
                                 bias=bia[0:P1, BC_EPS:BC_EPS + 1])

            def attention(b):
                s_sb = att.tile([P1, NT1, T2], BF, tag="s_sb")
                e_sb = att.tile([P1, NT1, T2], BF, tag="e_sb")
                sume = att.tile([P1, NT1], F32, tag="sume")
                lse = att.tile([P1, NT1], F32, tag="lse")
                obig = att.tile([P1, NT1, T2], BF, tag="obig")
                for g in range(4):
                    pst = pa.tile([P1, 2, 512], F32, tag="pa")
                    for j in range(2):
                        i = g * 2 + j
                        nc.tensor.matmul(pst[:, j, 0:T2],
                                         q_aug[:, b, i * P1:(i + 1) * P1],
                                         k_sb[:, b, :], start=True, stop=True)
                    nc.vector.tensor_copy(s_sb[:, 2 * g:2 * g + 2, :],
                                          pst[:, :, 0:T2])
                nc.scalar.activation(e_sb[:], s_sb[:], AF.Exp)
                nc.vector.reduce_sum(sume[:], e_sb[:], axis=AX.X)
                nc.scalar.activation(lse[:], sume[:], AF.Ln)
                for i in range(NT1):
                    nc.vector.scalar_tensor_tensor(
                        obig[:, i, :], in0=s_sb[:, i, :], scalar=lse[:, i:i + 1],
                        in1=lp[:, b, i, :], op0=ALU.subtract, op1=ALU.add)
                nc.sync.dma_start(out_l[b], obig[:])

            attention(0)
            query_encoder(1)
            attention(1)

    if fixup:
        _split_multi_waits(nc)
    return nc


_NC = None
_last_res = None


def _get_nc():
    global _NC
    if _NC is None:
        _NC = _build()
    return _NC


def _pack_weights(kw1, kb1, kw2, kb2, qw1, qb1, qw2, qb2, qw3, qb3):
    """Shared (per-core-independent) blob regions."""
    kw = np.zeros((128, K_C), BF16)
    kw[:, K_W1:K_W2] = (
        kw1.transpose(1, 2, 0).reshape(2, 128, 3, 512)
        .transpose(1, 2, 0, 3).reshape(128, 3072).astype(BF16))
    kw[:, K_W2:K_KX] = (
        kw2[:, :, 0].T.reshape(4, 128, 80).transpose(1, 0, 2)
        .reshape(128, 320).astype(BF16))

    qw = np.zeros((128, Q_C), BF16)
    qw[0:80, Q_W1:Q_W2] = (
        qw1.transpose(1, 2, 0).reshape(80, 480).astype(BF16))
    qw[0:80, Q_W2:Q_W3] = (
        qw2[:, :, 0].T.reshape(2, 80, 80).transpose(1, 0, 2)
        .reshape(80, 160).astype(BF16))
    qw[0:80, Q_W3:Q_W3 + 80] = (1e-3 * qw3[:, :, 0]).T.astype(BF16)

    bias = np.zeros((128, 11), np.float32)
    bias[:, BC_B1:BC_B1 + 4] = kb1.reshape(4, 128).T
    bias[0:80, BC_B1Q:BC_B1Q + 2] = qb1.reshape(2, 80).T
    bias[0:80, BC_B2] = kb2
    bias[0:80, BC_B2Q] = qb2
    bias[0:80, BC_B3Q] = 1e-3 * qb3
    bias[:, BC_ONES] = 1.0
    bias[:, BC_EPS] = 1e-8
    qw[:, Q_BIAS:Q_QX] = bias.view(BF16)
    return kw, qw


def _prep_core(queries, keys, prior, kw, qw):
    kblob = kw.copy()
    kxr = np.zeros((2, 128, BL, T2 + 2), BF16)
    kxr[:, :, :, 1:T2 + 1] = (
        keys.reshape(BL, 2, 128, T2).transpose(1, 2, 0, 3).astype(BF16))
    kblob[:, K_KX:K_C] = kxr.transpose(1, 0, 2, 3).reshape(128, 2 * BL * (T2 + 2))

    qblob = qw.copy()
    qxr = np.zeros((80, BL, T1 + 2), BF16)
    qxr[:, :, 1:T1 + 1] = queries.transpose(1, 0, 2).astype(BF16)
    qblob[0:80, Q_QX:Q_C] = qxr.reshape(80, BL * (T1 + 2))

    p_x = np.ascontiguousarray(
        prior.reshape(BL, NT1, P1, T2).transpose(2, 0, 1, 3).astype(BF16))
    return {"kblob": kblob, "qblob": qblob, "p_x": p_x}


def kernel(queries, keys, attn_prior, kw1, kb1, kw2, kb2,
           qw1, qb1, qw2, qb2, qw3, qb3):
    nc = _get_nc()
    kw, qw = _pack_weights(
        np.asarray(kw1, np.float32), np.asarray(kb1, np.float32),
        np.asarray(kw2, np.float32), np.asarray(kb2, np.float32),
        np.asarray(qw1, np.float32), np.asarray(qb1, np.float32),
        np.asarray(qw2, np.float32), np.asarray(qb2, np.float32),
        np.asarray(qw3, np.float32), np.asarray(qb3, np.float32))
    queries = np.asarray(queries, np.float32)
    keys = np.asarray(keys, np.float32)
    attn_prior = np.asarray(attn_prior, np.float32)

    in_maps = [
        _prep_core(queries[c * BL:(c + 1) * BL], keys[c * BL:(c + 1) * BL],
                   attn_prior[c * BL:(c + 1) * BL], kw, qw)
        for c in range(N_CORES)
    ]
    trace = bool(os.environ.get("CONVATTN_TRACE"))
    res = run_bass_kernel_spmd(nc, in_maps, core_ids=list(range(N_CORES)),
                               trace=trace)
    global _last_res
    _last_res = res

    full = np.empty((B, T1, T2), np.float32)
    for c in range(N_CORES):
        o = res.results[c]["out_l"]          # (BL, P1, NT1, T2) bf16
        full[c * BL:(c + 1) * BL] = (
            o.astype(np.float32).transpose(0, 2, 1, 3).reshape(BL, T1, T2))
    return full[:, None]
